# revision 19
# baseline (speedup 1.0000x reference)
"""Trainium2 Bass kernel for nn_MetaphorModel (masked segment-mean pool +
tiny linear classifier + CE loss).

Strategy (pure data parallel, 8 NeuronCores):
  - Shard batch B=256 across 8 cores (32 samples/core).
  - Only ~half the S=512 token rows are masked-in; the device gathers
    just those rows from HBM with indirect (gather) DMA, cutting HBM
    traffic ~2x vs a dense read. The mask's row set is decomposed on
    host into runs of 4/2/1 consecutive rows; each indirect DMA moves
    128 runs (one per SBUF partition). Longer runs mean fewer, larger
    DMA descriptors. Issue order round-robins across run sizes so SWDGE
    descriptor emission stays ahead of the SDMA engines.
  - The masked mean-pool is a matmul over the gathered rows: for each
    128-row chunk, lhsT is a [128, 32] "indicator" matrix whose column b
    holds 1/count[b] at rows belonging to sample b (0 elsewhere, 0 for
    pad rows). PSUM accumulates pooled [32, 768] across all chunks.
  - Classifier (768 -> 2) runs on the vector engine as two
    multiply+reduce ops against replicated weight rows, plus bias.
    Logits [32, 2] DMA'd out per core.
  - Host computes the scalar CE loss from the gathered [256, 2] logits
    (trivial epilogue) and returns (loss, logits) like the reference.

All host-side prep derives only from the tiny [256, 512] bool mask; the
384 MB of hidden states is touched exclusively by the device.
"""

import numpy as np

_B, _S, _D, _C = 256, 512, 768, 2
_NCORES = 8
_BPC = _B // _NCORES        # samples per core = 32
_P = 128                    # SBUF partitions
_SIZES = (4, 2, 1)          # gather run lengths (rows per descriptor)
_IND_SPLIT = 4              # indicator table loaded as this many tiles

LAST_RESULTS = None


def _issue_order(n_dmas):
    """Round-robin DMA issue order across run sizes: [(u, g), ...]."""
    order = []
    cnt = {u: 0 for u in _SIZES}
    while any(cnt[u] < n_dmas[u] for u in _SIZES):
        for u in _SIZES:
            if cnt[u] < n_dmas[u]:
                order.append((u, cnt[u]))
                cnt[u] += 1
    return order


def _build_program(n_dmas):
    """n_dmas: dict size -> number of gather DMAs of that run size."""
    from contextlib import ExitStack

    import concourse.bacc as bacc
    import concourse.bass as bass
    import concourse.mybir as mybir
    import concourse.tile as tile

    f32 = mybir.dt.float32
    f32r = mybir.dt.float32r
    i32 = mybir.dt.int32
    order = _issue_order(n_dmas)
    nch = sum(u for u, _ in order)
    csz = -(-nch // _IND_SPLIT)  # chunks per indicator tile

    nc = bacc.Bacc("TRN2", target_bir_lowering=False, debug=False,
                   num_devices=_NCORES)

    h_d = nc.dram_tensor("h", [_BPC * _S, _D], f32r, kind="ExternalInput")
    ix_d = {u: nc.dram_tensor(f"ix{u}", [_P, max(n_dmas[u], 1)], i32,
                              kind="ExternalInput") for u in _SIZES}
    ind_d = nc.dram_tensor("ind", [_P, nch * _BPC], f32r,
                           kind="ExternalInput")
    w_d = nc.dram_tensor("wrep", [_BPC, _C * _D], f32, kind="ExternalInput")
    b_d = nc.dram_tensor("brep", [_BPC, _C], f32, kind="ExternalInput")
    lg_d = nc.dram_tensor("logits", [_BPC, _C], f32, kind="ExternalOutput")

    with tile.TileContext(nc) as tc, ExitStack() as ctx:
        pools = {
            4: ctx.enter_context(tc.tile_pool(name="h4", bufs=6)),
            2: ctx.enter_context(tc.tile_pool(name="h2", bufs=10)),
            1: ctx.enter_context(tc.tile_pool(name="h1", bufs=12)),
        }
        cpool = ctx.enter_context(tc.tile_pool(name="const", bufs=1))
        pspool = ctx.enter_context(tc.tile_pool(name="ps", bufs=1,
                                                space="PSUM"))
        epool = ctx.enter_context(tc.tile_pool(name="ep", bufs=1))

        ix_sb = {}
        for u in _SIZES:
            ix_sb[u] = cpool.tile([_P, max(n_dmas[u], 1)], i32,
                                  name=f"ixsb{u}")
            nc.sync.dma_start(ix_sb[u][:], ix_d[u].ap())
        ind_sb = []
        for t in range(_IND_SPLIT):
            lo = t * csz * _BPC
            hi = min(nch, (t + 1) * csz) * _BPC
            tl = cpool.tile([_P, max(hi - lo, _BPC)], f32r, name=f"indsb{t}")
            if hi > lo:
                nc.sync.dma_start(tl[:, :hi - lo], ind_d.ap()[:, lo:hi])
            ind_sb.append(tl)
        wrep = cpool.tile([_BPC, _C * _D], f32)
        nc.sync.dma_start(wrep[:], w_d.ap())
        brep = cpool.tile([_BPC, _C], f32)
        nc.sync.dma_start(brep[:], b_d.ap())

        def ind_col(k):
            t, r = k // csz, k % csz
            return ind_sb[t][:, r * _BPC:(r + 1) * _BPC]

        # two accumulation epochs: epoch A's classifier reduce runs while
        # epoch B is still gathering, shrinking the kernel tail
        two_epochs = nch >= 2
        split = max(1, min(nch - 1, (2 * nch) // 3)) if two_epochs else nch
        pooled_a = pspool.tile([_BPC, _D], f32, name="pooled_a")
        pooled_b = pspool.tile([_BPC, _D], f32, name="pooled_b")
        half = {}  # (epoch, class) -> [32, 1] partial logits
        k = 0

        def classify(pooled_t, ep):
            for ci in range(_C):
                prod = epool.tile([_BPC, _D], f32, name=f"prod{ep}{ci}",
                                  tag=f"prod{ci}")
                nc.vector.tensor_mul(prod[:], pooled_t[:],
                                     wrep[:, ci * _D:(ci + 1) * _D])
                r = epool.tile([_BPC, 1], f32, name=f"r{ep}{ci}",
                               tag=f"r{ep}{ci}")
                nc.vector.reduce_sum(r[:], prod[:],
                                     axis=mybir.AxisListType.X)
                half[(ep, ci)] = r

        for u, g in order:
            ht = pools[u].tile([_P, u * _D], f32r, name=f"ht{u}",
                               tag=f"ht{u}")
            nc.gpsimd.indirect_dma_start(
                out=ht[:], out_offset=None, in_=h_d.ap()[:],
                in_offset=bass.IndirectOffsetOnAxis(
                    ap=ix_sb[u][:, g:g + 1], axis=0))
            for c in range(u):
                lhsT = ind_col(k)
                pooled = pooled_a if k < split else pooled_b
                first = k == 0 or k == split
                last = k == split - 1 or k == nch - 1
                nc.tensor.matmul(pooled[:, 0:512], lhsT,
                                 ht[:, c * _D:c * _D + 512],
                                 start=first, stop=last)
                nc.tensor.matmul(pooled[:, 512:_D], lhsT,
                                 ht[:, c * _D + 512:(c + 1) * _D],
                                 start=first, stop=last)
                k += 1
                if k == split:
                    classify(pooled_a, 0)
        if two_epochs:
            classify(pooled_b, 1)

        lg = epool.tile([_BPC, _C], f32)
        for ci in range(_C):
            if two_epochs:
                nc.vector.tensor_add(lg[:, ci:ci + 1], half[(0, ci)][:],
                                     half[(1, ci)][:])
            else:
                nc.vector.tensor_copy(lg[:, ci:ci + 1], half[(0, ci)][:])
        nc.vector.tensor_add(lg[:], lg[:], brep[:])
        nc.sync.dma_start(lg_d.ap(), lg[:])

    nc.compile()
    return nc


def _decompose_runs(mask_row):
    """Masked positions of one sample -> dict size -> list of run starts."""
    s = np.flatnonzero(mask_row)
    out = {u: [] for u in _SIZES}
    i = 0
    n = len(s)
    while i < n:
        j = i
        while j + 1 < n and s[j + 1] == s[j] + 1:
            j += 1
        L = j - i + 1  # maximal run s[i..j]
        pos = s[i]
        for u in _SIZES:
            while L >= u:
                out[u].append(pos)
                pos += u
                L -= u
        i = j + 1
    return out


def _capped_units(mask, budgets):
    """Per-core unit lists honoring per-size unit budgets: quads are taken
    while budget lasts, leftovers split into pairs, then singles.
    Returns dict (core, u) -> (rows, owners)."""
    units = {}
    for core in range(_NCORES):
        b0 = core * _BPC
        left = {u: budgets[u] for u in _SIZES}
        lists = {u: ([], []) for u in _SIZES}
        for b in range(_BPC):
            m = mask[b0 + b]
            s = np.flatnonzero(m)
            i, n = 0, len(s)
            while i < n:
                j = i
                while j + 1 < n and s[j + 1] == s[j] + 1:
                    j += 1
                L = j - i + 1
                pos = s[i]
                for u in _SIZES:
                    while L >= u and left[u] > 0:
                        lists[u][0].append(b * _S + pos)
                        lists[u][1].append(b)
                        left[u] -= 1
                        pos += u
                        L -= u
                assert L == 0, "unit budgets infeasible"
                i = j + 1
        for u in _SIZES:
            units[(core, u)] = (np.asarray(lists[u][0], np.int32),
                                np.asarray(lists[u][1], np.int64))
    return units


def _search_ndmas(mask):
    """Smallest-capacity feasible DMA-count vector across all cores.

    Feasibility per core from maximal-decomposition unit counts
    (u4, u2, u1): with q = min(u4, cap4) quads, excess quads become 2
    pairs each; with p = min(pairs, cap2), excess pairs become 2 singles
    each; need remaining singles <= cap1."""
    per_core = []
    for core in range(_NCORES):
        b0 = core * _BPC
        u = {s: 0 for s in _SIZES}
        for b in range(_BPC):
            r = _decompose_runs(mask[b0 + b])
            for s in _SIZES:
                u[s] += len(r[s])
        per_core.append(u)

    def feasible(n4, n2, n1):
        for u in per_core:
            q = min(u[4], n4 * _P)
            pairs = u[2] + 2 * (u[4] - q)
            p = min(pairs, n2 * _P)
            singles = u[1] + 2 * (pairs - p)
            if singles > n1 * _P:
                return False
        return True

    best = None
    max4 = max(-(-u[4] // _P) for u in per_core) + 1
    for n4 in range(0, max4 + 1):
        for n2 in range(0, 30):
            # smallest feasible n1 for this (n4, n2)
            need1 = 0
            for u in per_core:
                q = min(u[4], n4 * _P)
                pairs = u[2] + 2 * (u[4] - q)
                p = min(pairs, n2 * _P)
                need1 = max(need1, u[1] + 2 * (pairs - p))
            n1 = -(-need1 // _P)
            cap = 4 * n4 + 2 * n2 + n1
            key = (cap, n4 + n2 + n1)
            if best is None or key < best[0]:
                if feasible(n4, n2, n1):
                    best = (key, {4: n4, 2: n2, 1: max(n1, 1)})
    n = best[1]
    return {u: max(n[u], 1) for u in _SIZES}


def _prep(h, mask, W, bias):
    """Returns (n_dmas, in_maps)."""
    maskf = mask.astype(np.float32)
    counts = maskf.sum(axis=1)                      # [B]

    n_dmas = _search_ndmas(mask)
    units = _capped_units(mask, {u: n_dmas[u] * _P for u in _SIZES})
    order = _issue_order(n_dmas)
    nch = sum(u for u, _ in order)

    # chunk base index of each DMA in issue order
    cbase = {}
    k = 0
    for u, g in order:
        cbase[(u, g)] = k
        k += u

    wrep_np = np.ascontiguousarray(
        np.broadcast_to(W.reshape(1, _C * _D), (_BPC, _C * _D)))
    brep_np = np.ascontiguousarray(
        np.broadcast_to(bias.reshape(1, _C), (_BPC, _C)))

    in_maps = []
    for core in range(_NCORES):
        b0 = core * _BPC
        hc = np.ascontiguousarray(h[b0:b0 + _BPC].reshape(_BPC * _S, _D))
        im = {"h": hc, "wrep": wrep_np, "brep": brep_np}
        ind_np = np.zeros((_P, nch * _BPC), np.float32)
        for u in _SIZES:
            rows, owners = units[(core, u)]
            n = n_dmas[u]
            ix_np = np.zeros((_P, n), np.int32)
            nu = len(rows)
            if nu:
                w = (1.0 / counts[b0 + owners]).astype(np.float32)
                for i in range(nu):
                    g, p = divmod(i, _P)
                    ix_np[p, g] = rows[i]
                    kb = cbase[(u, g)]
                    for c in range(u):
                        ind_np[p, (kb + c) * _BPC + owners[i]] = w[i]
            im[f"ix{u}"] = np.ascontiguousarray(ix_np)
        im["ind"] = ind_np
        in_maps.append(im)
    return n_dmas, in_maps


def _ensure_axon_hooks_shim():
    """bass_utils imports antenv.axon_hooks on its trace path; some
    containers lack that module. Register a no-op shim so a stray
    BASS_TRACE=1 in the environment degrades to an untraced run."""
    import sys
    import types
    try:
        import antenv.axon_hooks  # noqa: F401
    except Exception:
        try:
            m = types.ModuleType("antenv.axon_hooks")
            m.set_axon_ntff_profile_hook = lambda h: None
            m.get_axon_ntff_profile_hook = lambda: None
            sys.modules["antenv.axon_hooks"] = m
        except Exception:
            pass


def kernel(last_hidden_state, metaphor_mask, labels, classifier_w,
           classifier_b):
    global LAST_RESULTS
    _ensure_axon_hooks_shim()
    from concourse.bass_utils import run_bass_kernel_spmd

    h = np.asarray(last_hidden_state, dtype=np.float32)
    mask = np.asarray(metaphor_mask).astype(bool)
    labels = np.asarray(labels)
    W = np.asarray(classifier_w, dtype=np.float32)
    bias = np.asarray(classifier_b, dtype=np.float32)

    assert h.shape == (_B, _S, _D) and W.shape == (_C, _D)

    n_dmas, in_maps = _prep(h, mask, W, bias)
    nc = _build_program(n_dmas)

    res = run_bass_kernel_spmd(nc, in_maps, core_ids=list(range(_NCORES)))
    LAST_RESULTS = res
    logits = np.concatenate([r["logits"] for r in res.results], axis=0)

    # Host epilogue: CE loss (mean reduction) over the tiny [256, 2] logits.
    lg64 = logits.astype(np.float64)
    m = lg64.max(axis=1, keepdims=True)
    lse = (m[:, 0] + np.log(np.exp(lg64 - m).sum(axis=1)))
    nll = lse - lg64[np.arange(_B), labels.astype(np.int64)]
    loss = np.float32(nll.mean())
    return loss, logits


# revision 21
# speedup vs baseline: 1.0027x; 1.0027x over previous
"""Trainium2 Bass kernel for nn_MetaphorModel (masked segment-mean pool +
tiny linear classifier + CE loss).

Strategy (pure data parallel, 8 NeuronCores):
  - Shard batch B=256 across 8 cores (32 samples/core).
  - Only ~half the S=512 token rows are masked-in; the device gathers
    just those rows from HBM with indirect (gather) DMA, cutting HBM
    traffic ~2x vs a dense read. The mask's row set is decomposed on
    host into runs of 4/2/1 consecutive rows; each indirect DMA moves
    128 runs (one per SBUF partition). Longer runs mean fewer, larger
    DMA descriptors. Issue order round-robins across run sizes so SWDGE
    descriptor emission stays ahead of the SDMA engines.
  - The masked mean-pool is a matmul over the gathered rows: for each
    128-row chunk, lhsT is a [128, 32] "indicator" matrix whose column b
    holds 1/count[b] at rows belonging to sample b (0 elsewhere, 0 for
    pad rows). PSUM accumulates pooled [32, 768] across all chunks.
  - Classifier (768 -> 2) runs on the vector engine as two
    multiply+reduce ops against replicated weight rows, plus bias.
    Logits [32, 2] DMA'd out per core.
  - Host computes the scalar CE loss from the gathered [256, 2] logits
    (trivial epilogue) and returns (loss, logits) like the reference.

All host-side prep derives only from the tiny [256, 512] bool mask; the
384 MB of hidden states is touched exclusively by the device.
"""

import numpy as np

_B, _S, _D, _C = 256, 512, 768, 2
_NCORES = 8
_BPC = _B // _NCORES        # samples per core = 32
_P = 128                    # SBUF partitions
_SIZES = (4, 2, 1)          # gather run lengths (rows per descriptor)
_IND_SPLIT = 4              # indicator table loaded as this many tiles

LAST_RESULTS = None


def _issue_order(n_dmas):
    """Round-robin DMA issue order across run sizes: [(u, g), ...].

    Measured best on HW: a balanced mix keeps SWDGE emission ahead of
    the SDMA engines throughout (descending-size and deadline-merge
    orders both measured slower — their long all-singles stretches are
    emission-starved)."""
    order = []
    cnt = {u: 0 for u in _SIZES}
    while any(cnt[u] < n_dmas[u] for u in _SIZES):
        for u in _SIZES:
            if cnt[u] < n_dmas[u]:
                order.append((u, cnt[u]))
                cnt[u] += 1
    return order


def _build_program(n_dmas):
    """n_dmas: dict size -> number of gather DMAs of that run size."""
    from contextlib import ExitStack

    import concourse.bacc as bacc
    import concourse.bass as bass
    import concourse.mybir as mybir
    import concourse.tile as tile

    f32 = mybir.dt.float32
    f32r = mybir.dt.float32r
    i32 = mybir.dt.int32
    order = _issue_order(n_dmas)
    nch = sum(u for u, _ in order)
    csz = -(-nch // _IND_SPLIT)  # chunks per indicator tile

    nc = bacc.Bacc("TRN2", target_bir_lowering=False, debug=False,
                   num_devices=_NCORES)

    h_d = nc.dram_tensor("h", [_BPC * _S, _D], f32r, kind="ExternalInput")
    ix_d = {u: nc.dram_tensor(f"ix{u}", [_P, max(n_dmas[u], 1)], i32,
                              kind="ExternalInput") for u in _SIZES}
    ind_d = nc.dram_tensor("ind", [_P, nch * _BPC], f32r,
                           kind="ExternalInput")
    w_d = nc.dram_tensor("wrep", [_BPC, _C * _D], f32, kind="ExternalInput")
    b_d = nc.dram_tensor("brep", [_BPC, _C], f32, kind="ExternalInput")
    lg_d = nc.dram_tensor("logits", [_BPC, _C], f32, kind="ExternalOutput")

    with tile.TileContext(nc) as tc, ExitStack() as ctx:
        pools = {
            4: ctx.enter_context(tc.tile_pool(name="h4", bufs=6)),
            2: ctx.enter_context(tc.tile_pool(name="h2", bufs=10)),
            1: ctx.enter_context(tc.tile_pool(name="h1", bufs=12)),
        }
        cpool = ctx.enter_context(tc.tile_pool(name="const", bufs=1))
        pspool = ctx.enter_context(tc.tile_pool(name="ps", bufs=1,
                                                space="PSUM"))
        epool = ctx.enter_context(tc.tile_pool(name="ep", bufs=1))

        ix_sb = {}
        for u in _SIZES:
            ix_sb[u] = cpool.tile([_P, max(n_dmas[u], 1)], i32,
                                  name=f"ixsb{u}")
            nc.sync.dma_start(ix_sb[u][:], ix_d[u].ap())
        ind_sb = []
        for t in range(_IND_SPLIT):
            lo = t * csz * _BPC
            hi = min(nch, (t + 1) * csz) * _BPC
            tl = cpool.tile([_P, max(hi - lo, _BPC)], f32r, name=f"indsb{t}")
            if hi > lo:
                nc.sync.dma_start(tl[:, :hi - lo], ind_d.ap()[:, lo:hi])
            ind_sb.append(tl)
        wrep = cpool.tile([_BPC, _C * _D], f32)
        nc.sync.dma_start(wrep[:], w_d.ap())
        brep = cpool.tile([_BPC, _C], f32)
        nc.sync.dma_start(brep[:], b_d.ap())

        def ind_col(k):
            t, r = k // csz, k % csz
            return ind_sb[t][:, r * _BPC:(r + 1) * _BPC]

        # two accumulation epochs: epoch A's classifier reduce runs while
        # epoch B is still gathering, shrinking the kernel tail
        two_epochs = nch >= 2
        split = max(1, min(nch - 1, (2 * nch) // 3)) if two_epochs else nch
        pooled_a = pspool.tile([_BPC, _D], f32, name="pooled_a")
        pooled_b = pspool.tile([_BPC, _D], f32, name="pooled_b")
        half = {}  # (epoch, class) -> [32, 1] partial logits
        k = 0

        def classify(pooled_t, ep):
            for ci in range(_C):
                prod = epool.tile([_BPC, _D], f32, name=f"prod{ep}{ci}",
                                  tag=f"prod{ci}")
                nc.vector.tensor_mul(prod[:], pooled_t[:],
                                     wrep[:, ci * _D:(ci + 1) * _D])
                r = epool.tile([_BPC, 1], f32, name=f"r{ep}{ci}",
                               tag=f"r{ep}{ci}")
                nc.vector.reduce_sum(r[:], prod[:],
                                     axis=mybir.AxisListType.X)
                half[(ep, ci)] = r

        for u, g in order:
            ht = pools[u].tile([_P, u * _D], f32r, name=f"ht{u}",
                               tag=f"ht{u}")
            nc.gpsimd.indirect_dma_start(
                out=ht[:], out_offset=None, in_=h_d.ap()[:],
                in_offset=bass.IndirectOffsetOnAxis(
                    ap=ix_sb[u][:, g:g + 1], axis=0))
            for c in range(u):
                lhsT = ind_col(k)
                pooled = pooled_a if k < split else pooled_b
                first = k == 0 or k == split
                last = k == split - 1 or k == nch - 1
                nc.tensor.matmul(pooled[:, 0:512], lhsT,
                                 ht[:, c * _D:c * _D + 512],
                                 start=first, stop=last)
                nc.tensor.matmul(pooled[:, 512:_D], lhsT,
                                 ht[:, c * _D + 512:(c + 1) * _D],
                                 start=first, stop=last)
                k += 1
                if k == split:
                    classify(pooled_a, 0)
        if two_epochs:
            classify(pooled_b, 1)

        lg = epool.tile([_BPC, _C], f32)
        for ci in range(_C):
            if two_epochs:
                nc.vector.tensor_add(lg[:, ci:ci + 1], half[(0, ci)][:],
                                     half[(1, ci)][:])
            else:
                nc.vector.tensor_copy(lg[:, ci:ci + 1], half[(0, ci)][:])
        nc.vector.tensor_add(lg[:], lg[:], brep[:])
        nc.sync.dma_start(lg_d.ap(), lg[:])

    nc.compile()
    return nc


def _decompose_runs(mask_row):
    """Masked positions of one sample -> dict size -> list of run starts."""
    s = np.flatnonzero(mask_row)
    out = {u: [] for u in _SIZES}
    i = 0
    n = len(s)
    while i < n:
        j = i
        while j + 1 < n and s[j + 1] == s[j] + 1:
            j += 1
        L = j - i + 1  # maximal run s[i..j]
        pos = s[i]
        for u in _SIZES:
            while L >= u:
                out[u].append(pos)
                pos += u
                L -= u
        i = j + 1
    return out


def _capped_units(mask, budgets):
    """Per-core unit lists honoring per-size unit budgets: quads are taken
    while budget lasts, leftovers split into pairs, then singles.
    Returns dict (core, u) -> (rows, owners)."""
    units = {}
    for core in range(_NCORES):
        b0 = core * _BPC
        left = {u: budgets[u] for u in _SIZES}
        lists = {u: ([], []) for u in _SIZES}
        for b in range(_BPC):
            m = mask[b0 + b]
            s = np.flatnonzero(m)
            i, n = 0, len(s)
            while i < n:
                j = i
                while j + 1 < n and s[j + 1] == s[j] + 1:
                    j += 1
                L = j - i + 1
                pos = s[i]
                for u in _SIZES:
                    while L >= u and left[u] > 0:
                        lists[u][0].append(b * _S + pos)
                        lists[u][1].append(b)
                        left[u] -= 1
                        pos += u
                        L -= u
                assert L == 0, "unit budgets infeasible"
                i = j + 1
        for u in _SIZES:
            units[(core, u)] = (np.asarray(lists[u][0], np.int32),
                                np.asarray(lists[u][1], np.int64))
    return units


def _search_ndmas(mask):
    """Smallest-capacity feasible DMA-count vector across all cores.

    Feasibility per core from maximal-decomposition unit counts
    (u4, u2, u1): with q = min(u4, cap4) quads, excess quads become 2
    pairs each; with p = min(pairs, cap2), excess pairs become 2 singles
    each; need remaining singles <= cap1."""
    per_core = []
    for core in range(_NCORES):
        b0 = core * _BPC
        u = {s: 0 for s in _SIZES}
        for b in range(_BPC):
            r = _decompose_runs(mask[b0 + b])
            for s in _SIZES:
                u[s] += len(r[s])
        per_core.append(u)

    def feasible(n4, n2, n1):
        for u in per_core:
            q = min(u[4], n4 * _P)
            pairs = u[2] + 2 * (u[4] - q)
            p = min(pairs, n2 * _P)
            singles = u[1] + 2 * (pairs - p)
            if singles > n1 * _P:
                return False
        return True

    best = None
    max4 = max(-(-u[4] // _P) for u in per_core) + 1
    for n4 in range(0, max4 + 1):
        for n2 in range(0, 30):
            # smallest feasible n1 for this (n4, n2)
            need1 = 0
            for u in per_core:
                q = min(u[4], n4 * _P)
                pairs = u[2] + 2 * (u[4] - q)
                p = min(pairs, n2 * _P)
                need1 = max(need1, u[1] + 2 * (pairs - p))
            n1 = -(-need1 // _P)
            cap = 4 * n4 + 2 * n2 + n1
            key = (cap, n4 + n2 + n1)
            if best is None or key < best[0]:
                if feasible(n4, n2, n1):
                    best = (key, {4: n4, 2: n2, 1: max(n1, 1)})
    n = best[1]
    return {u: max(n[u], 1) for u in _SIZES}


def _prep(h, mask, W, bias):
    """Returns (n_dmas, in_maps)."""
    maskf = mask.astype(np.float32)
    counts = maskf.sum(axis=1)                      # [B]

    n_dmas = _search_ndmas(mask)
    units = _capped_units(mask, {u: n_dmas[u] * _P for u in _SIZES})
    order = _issue_order(n_dmas)
    nch = sum(u for u, _ in order)

    # chunk base index of each DMA in issue order
    cbase = {}
    k = 0
    for u, g in order:
        cbase[(u, g)] = k
        k += u

    wrep_np = np.ascontiguousarray(
        np.broadcast_to(W.reshape(1, _C * _D), (_BPC, _C * _D)))
    brep_np = np.ascontiguousarray(
        np.broadcast_to(bias.reshape(1, _C), (_BPC, _C)))

    in_maps = []
    for core in range(_NCORES):
        b0 = core * _BPC
        hc = np.ascontiguousarray(h[b0:b0 + _BPC].reshape(_BPC * _S, _D))
        im = {"h": hc, "wrep": wrep_np, "brep": brep_np}
        ind_np = np.zeros((_P, nch * _BPC), np.float32)
        for u in _SIZES:
            rows, owners = units[(core, u)]
            n = n_dmas[u]
            ix_np = np.zeros((_P, n), np.int32)
            nu = len(rows)
            if nu:
                w = (1.0 / counts[b0 + owners]).astype(np.float32)
                for i in range(nu):
                    g, p = divmod(i, _P)
                    ix_np[p, g] = rows[i]
                    kb = cbase[(u, g)]
                    for c in range(u):
                        ind_np[p, (kb + c) * _BPC + owners[i]] = w[i]
            im[f"ix{u}"] = np.ascontiguousarray(ix_np)
        im["ind"] = ind_np
        in_maps.append(im)
    return n_dmas, in_maps


def _ensure_axon_hooks_shim():
    """bass_utils imports antenv.axon_hooks on its trace path; some
    containers lack that module. Register a no-op shim so a stray
    BASS_TRACE=1 in the environment degrades to an untraced run."""
    import sys
    import types
    try:
        import antenv.axon_hooks  # noqa: F401
    except Exception:
        try:
            m = types.ModuleType("antenv.axon_hooks")
            m.set_axon_ntff_profile_hook = lambda h: None
            m.get_axon_ntff_profile_hook = lambda: None
            sys.modules["antenv.axon_hooks"] = m
        except Exception:
            pass


def kernel(last_hidden_state, metaphor_mask, labels, classifier_w,
           classifier_b):
    global LAST_RESULTS
    _ensure_axon_hooks_shim()
    from concourse.bass_utils import run_bass_kernel_spmd

    h = np.asarray(last_hidden_state, dtype=np.float32)
    mask = np.asarray(metaphor_mask).astype(bool)
    labels = np.asarray(labels)
    W = np.asarray(classifier_w, dtype=np.float32)
    bias = np.asarray(classifier_b, dtype=np.float32)

    assert h.shape == (_B, _S, _D) and W.shape == (_C, _D)

    n_dmas, in_maps = _prep(h, mask, W, bias)
    nc = _build_program(n_dmas)

    res = run_bass_kernel_spmd(nc, in_maps, core_ids=list(range(_NCORES)))
    LAST_RESULTS = res
    logits = np.concatenate([r["logits"] for r in res.results], axis=0)

    # Host epilogue: CE loss (mean reduction) over the tiny [256, 2] logits.
    lg64 = logits.astype(np.float64)
    m = lg64.max(axis=1, keepdims=True)
    lse = (m[:, 0] + np.log(np.exp(lg64 - m).sum(axis=1)))
    nll = lse - lg64[np.arange(_B), labels.astype(np.int64)]
    loss = np.float32(nll.mean())
    return loss, logits


# revision 22
# speedup vs baseline: 1.0127x; 1.0099x over previous
"""Trainium2 Bass kernel for nn_MetaphorModel (masked segment-mean pool +
tiny linear classifier + CE loss).

Strategy (pure data parallel, 8 NeuronCores):
  - Shard batch B=256 across 8 cores (32 samples/core).
  - Only ~half the S=512 token rows are masked-in; the device gathers
    just those rows from HBM with indirect (gather) DMA, cutting HBM
    traffic ~2x vs a dense read. The mask's row set is decomposed on
    host into runs of 4/2/1 consecutive rows; each indirect DMA moves
    128 runs (one per SBUF partition). Longer runs mean fewer, larger
    DMA descriptors. Issue order round-robins across run sizes so SWDGE
    descriptor emission stays ahead of the SDMA engines.
  - The masked mean-pool is a matmul over the gathered rows: for each
    128-row chunk, lhsT is a [128, 32] "indicator" matrix whose column b
    holds 1/count[b] at rows belonging to sample b (0 elsewhere, 0 for
    pad rows). PSUM accumulates pooled [32, 768] across all chunks.
  - Classifier (768 -> 2) runs on the vector engine as two
    multiply+reduce ops against replicated weight rows, plus bias.
    Logits [32, 2] DMA'd out per core.
  - Host computes the scalar CE loss from the gathered [256, 2] logits
    (trivial epilogue) and returns (loss, logits) like the reference.

All host-side prep derives only from the tiny [256, 512] bool mask; the
384 MB of hidden states is touched exclusively by the device.
"""

import numpy as np

_B, _S, _D, _C = 256, 512, 768, 2
_NCORES = 8
_BPC = _B // _NCORES        # samples per core = 32
_P = 128                    # SBUF partitions
_SIZES = (4, 2, 1)          # gather run lengths (rows per descriptor)
_IND_SPLIT = 4              # indicator table loaded as this many tiles

LAST_RESULTS = None


def _issue_order(n_dmas):
    """Round-robin DMA issue order across run sizes: [(u, g), ...].

    Measured best on HW: a balanced mix keeps SWDGE emission ahead of
    the SDMA engines throughout (descending-size and deadline-merge
    orders both measured slower — their long all-singles stretches are
    emission-starved)."""
    order = []
    cnt = {u: 0 for u in _SIZES}
    while any(cnt[u] < n_dmas[u] for u in _SIZES):
        for u in _SIZES:
            if cnt[u] < n_dmas[u]:
                order.append((u, cnt[u]))
                cnt[u] += 1
    return order


def _build_program(n_dmas):
    """n_dmas: dict size -> number of gather DMAs of that run size."""
    from contextlib import ExitStack

    import concourse.bacc as bacc
    import concourse.bass as bass
    import concourse.mybir as mybir
    import concourse.tile as tile

    f32 = mybir.dt.float32
    f32r = mybir.dt.float32r
    i32 = mybir.dt.int32
    order = _issue_order(n_dmas)
    nch = sum(u for u, _ in order)
    csz = -(-nch // _IND_SPLIT)  # chunks per indicator tile

    nc = bacc.Bacc("TRN2", target_bir_lowering=False, debug=False,
                   num_devices=_NCORES)

    h_d = nc.dram_tensor("h", [_BPC * _S, _D], f32r, kind="ExternalInput")
    ix_d = {u: nc.dram_tensor(f"ix{u}", [_P, max(n_dmas[u], 1)], i32,
                              kind="ExternalInput") for u in _SIZES}
    ind_d = nc.dram_tensor("ind", [_P, nch * _BPC], f32r,
                           kind="ExternalInput")
    w_d = nc.dram_tensor("wrep", [_BPC, _C * _D], f32, kind="ExternalInput")
    b_d = nc.dram_tensor("brep", [_BPC, _C], f32, kind="ExternalInput")
    lg_d = nc.dram_tensor("logits", [_BPC, _C], f32, kind="ExternalOutput")

    with tile.TileContext(nc) as tc, ExitStack() as ctx:
        pools = {
            4: ctx.enter_context(tc.tile_pool(name="h4", bufs=4)),
            2: ctx.enter_context(tc.tile_pool(name="h2", bufs=12)),
            1: ctx.enter_context(tc.tile_pool(name="h1", bufs=16)),
        }
        cpool = ctx.enter_context(tc.tile_pool(name="const", bufs=1))
        pspool = ctx.enter_context(tc.tile_pool(name="ps", bufs=1,
                                                space="PSUM"))
        epool = ctx.enter_context(tc.tile_pool(name="ep", bufs=1))

        ix_sb = {}
        for u in _SIZES:
            ix_sb[u] = cpool.tile([_P, max(n_dmas[u], 1)], i32,
                                  name=f"ixsb{u}")
            nc.sync.dma_start(ix_sb[u][:], ix_d[u].ap())
        ind_sb = []
        for t in range(_IND_SPLIT):
            lo = t * csz * _BPC
            hi = min(nch, (t + 1) * csz) * _BPC
            tl = cpool.tile([_P, max(hi - lo, _BPC)], f32r, name=f"indsb{t}")
            if hi > lo:
                nc.sync.dma_start(tl[:, :hi - lo], ind_d.ap()[:, lo:hi])
            ind_sb.append(tl)
        wrep = cpool.tile([_BPC, _C * _D], f32)
        nc.sync.dma_start(wrep[:], w_d.ap())
        brep = cpool.tile([_BPC, _C], f32)
        nc.sync.dma_start(brep[:], b_d.ap())

        def ind_col(k):
            t, r = k // csz, k % csz
            return ind_sb[t][:, r * _BPC:(r + 1) * _BPC]

        # two accumulation epochs: epoch A's classifier reduce runs while
        # epoch B is still gathering, shrinking the kernel tail
        two_epochs = nch >= 2
        split = max(1, min(nch - 1, (2 * nch) // 3)) if two_epochs else nch
        pooled_a = pspool.tile([_BPC, _D], f32, name="pooled_a")
        pooled_b = pspool.tile([_BPC, _D], f32, name="pooled_b")
        half = {}  # (epoch, class) -> [32, 1] partial logits
        k = 0

        def classify(pooled_t, ep):
            for ci in range(_C):
                prod = epool.tile([_BPC, _D], f32, name=f"prod{ep}{ci}",
                                  tag=f"prod{ci}")
                nc.vector.tensor_mul(prod[:], pooled_t[:],
                                     wrep[:, ci * _D:(ci + 1) * _D])
                r = epool.tile([_BPC, 1], f32, name=f"r{ep}{ci}",
                               tag=f"r{ep}{ci}")
                nc.vector.reduce_sum(r[:], prod[:],
                                     axis=mybir.AxisListType.X)
                half[(ep, ci)] = r

        for u, g in order:
            ht = pools[u].tile([_P, u * _D], f32r, name=f"ht{u}",
                               tag=f"ht{u}")
            nc.gpsimd.indirect_dma_start(
                out=ht[:], out_offset=None, in_=h_d.ap()[:],
                in_offset=bass.IndirectOffsetOnAxis(
                    ap=ix_sb[u][:, g:g + 1], axis=0))
            for c in range(u):
                lhsT = ind_col(k)
                pooled = pooled_a if k < split else pooled_b
                first = k == 0 or k == split
                last = k == split - 1 or k == nch - 1
                nc.tensor.matmul(pooled[:, 0:512], lhsT,
                                 ht[:, c * _D:c * _D + 512],
                                 start=first, stop=last)
                nc.tensor.matmul(pooled[:, 512:_D], lhsT,
                                 ht[:, c * _D + 512:(c + 1) * _D],
                                 start=first, stop=last)
                k += 1
                if k == split:
                    classify(pooled_a, 0)
        if two_epochs:
            classify(pooled_b, 1)

        lg = epool.tile([_BPC, _C], f32)
        for ci in range(_C):
            if two_epochs:
                nc.vector.tensor_add(lg[:, ci:ci + 1], half[(0, ci)][:],
                                     half[(1, ci)][:])
            else:
                nc.vector.tensor_copy(lg[:, ci:ci + 1], half[(0, ci)][:])
        nc.vector.tensor_add(lg[:], lg[:], brep[:])
        nc.sync.dma_start(lg_d.ap(), lg[:])

    nc.compile()
    return nc


def _decompose_runs(mask_row):
    """Masked positions of one sample -> dict size -> list of run starts."""
    s = np.flatnonzero(mask_row)
    out = {u: [] for u in _SIZES}
    i = 0
    n = len(s)
    while i < n:
        j = i
        while j + 1 < n and s[j + 1] == s[j] + 1:
            j += 1
        L = j - i + 1  # maximal run s[i..j]
        pos = s[i]
        for u in _SIZES:
            while L >= u:
                out[u].append(pos)
                pos += u
                L -= u
        i = j + 1
    return out


def _capped_units(mask, budgets):
    """Per-core unit lists honoring per-size unit budgets: quads are taken
    while budget lasts, leftovers split into pairs, then singles.
    Returns dict (core, u) -> (rows, owners)."""
    units = {}
    for core in range(_NCORES):
        b0 = core * _BPC
        left = {u: budgets[u] for u in _SIZES}
        lists = {u: ([], []) for u in _SIZES}
        for b in range(_BPC):
            m = mask[b0 + b]
            s = np.flatnonzero(m)
            i, n = 0, len(s)
            while i < n:
                j = i
                while j + 1 < n and s[j + 1] == s[j] + 1:
                    j += 1
                L = j - i + 1
                pos = s[i]
                for u in _SIZES:
                    while L >= u and left[u] > 0:
                        lists[u][0].append(b * _S + pos)
                        lists[u][1].append(b)
                        left[u] -= 1
                        pos += u
                        L -= u
                assert L == 0, "unit budgets infeasible"
                i = j + 1
        for u in _SIZES:
            units[(core, u)] = (np.asarray(lists[u][0], np.int32),
                                np.asarray(lists[u][1], np.int64))
    return units


def _search_ndmas(mask):
    """Smallest-capacity feasible DMA-count vector across all cores.

    Feasibility per core from maximal-decomposition unit counts
    (u4, u2, u1): with q = min(u4, cap4) quads, excess quads become 2
    pairs each; with p = min(pairs, cap2), excess pairs become 2 singles
    each; need remaining singles <= cap1."""
    per_core = []
    for core in range(_NCORES):
        b0 = core * _BPC
        u = {s: 0 for s in _SIZES}
        for b in range(_BPC):
            r = _decompose_runs(mask[b0 + b])
            for s in _SIZES:
                u[s] += len(r[s])
        per_core.append(u)

    def feasible(n4, n2, n1):
        for u in per_core:
            q = min(u[4], n4 * _P)
            pairs = u[2] + 2 * (u[4] - q)
            p = min(pairs, n2 * _P)
            singles = u[1] + 2 * (pairs - p)
            if singles > n1 * _P:
                return False
        return True

    best = None
    max4 = max(-(-u[4] // _P) for u in per_core) + 1
    for n4 in range(0, max4 + 1):
        for n2 in range(0, 30):
            # smallest feasible n1 for this (n4, n2)
            need1 = 0
            for u in per_core:
                q = min(u[4], n4 * _P)
                pairs = u[2] + 2 * (u[4] - q)
                p = min(pairs, n2 * _P)
                need1 = max(need1, u[1] + 2 * (pairs - p))
            n1 = -(-need1 // _P)
            cap = 4 * n4 + 2 * n2 + n1
            key = (cap, n4 + n2 + n1)
            if best is None or key < best[0]:
                if feasible(n4, n2, n1):
                    best = (key, {4: n4, 2: n2, 1: max(n1, 1)})
    n = best[1]
    return {u: max(n[u], 1) for u in _SIZES}


def _prep(h, mask, W, bias):
    """Returns (n_dmas, in_maps)."""
    maskf = mask.astype(np.float32)
    counts = maskf.sum(axis=1)                      # [B]

    n_dmas = _search_ndmas(mask)
    units = _capped_units(mask, {u: n_dmas[u] * _P for u in _SIZES})
    order = _issue_order(n_dmas)
    nch = sum(u for u, _ in order)

    # chunk base index of each DMA in issue order
    cbase = {}
    k = 0
    for u, g in order:
        cbase[(u, g)] = k
        k += u

    wrep_np = np.ascontiguousarray(
        np.broadcast_to(W.reshape(1, _C * _D), (_BPC, _C * _D)))
    brep_np = np.ascontiguousarray(
        np.broadcast_to(bias.reshape(1, _C), (_BPC, _C)))

    in_maps = []
    for core in range(_NCORES):
        b0 = core * _BPC
        hc = np.ascontiguousarray(h[b0:b0 + _BPC].reshape(_BPC * _S, _D))
        im = {"h": hc, "wrep": wrep_np, "brep": brep_np}
        ind_np = np.zeros((_P, nch * _BPC), np.float32)
        for u in _SIZES:
            rows, owners = units[(core, u)]
            n = n_dmas[u]
            ix_np = np.zeros((_P, n), np.int32)
            nu = len(rows)
            if nu:
                w = (1.0 / counts[b0 + owners]).astype(np.float32)
                for i in range(nu):
                    g, p = divmod(i, _P)
                    ix_np[p, g] = rows[i]
                    kb = cbase[(u, g)]
                    for c in range(u):
                        ind_np[p, (kb + c) * _BPC + owners[i]] = w[i]
            im[f"ix{u}"] = np.ascontiguousarray(ix_np)
        im["ind"] = ind_np
        in_maps.append(im)
    return n_dmas, in_maps


def _ensure_axon_hooks_shim():
    """bass_utils imports antenv.axon_hooks on its trace path; some
    containers lack that module. Register a no-op shim so a stray
    BASS_TRACE=1 in the environment degrades to an untraced run."""
    import sys
    import types
    try:
        import antenv.axon_hooks  # noqa: F401
    except Exception:
        try:
            m = types.ModuleType("antenv.axon_hooks")
            m.set_axon_ntff_profile_hook = lambda h: None
            m.get_axon_ntff_profile_hook = lambda: None
            sys.modules["antenv.axon_hooks"] = m
        except Exception:
            pass


def kernel(last_hidden_state, metaphor_mask, labels, classifier_w,
           classifier_b):
    global LAST_RESULTS
    _ensure_axon_hooks_shim()
    from concourse.bass_utils import run_bass_kernel_spmd

    h = np.asarray(last_hidden_state, dtype=np.float32)
    mask = np.asarray(metaphor_mask).astype(bool)
    labels = np.asarray(labels)
    W = np.asarray(classifier_w, dtype=np.float32)
    bias = np.asarray(classifier_b, dtype=np.float32)

    assert h.shape == (_B, _S, _D) and W.shape == (_C, _D)

    n_dmas, in_maps = _prep(h, mask, W, bias)
    nc = _build_program(n_dmas)

    res = run_bass_kernel_spmd(nc, in_maps, core_ids=list(range(_NCORES)))
    LAST_RESULTS = res
    logits = np.concatenate([r["logits"] for r in res.results], axis=0)

    # Host epilogue: CE loss (mean reduction) over the tiny [256, 2] logits.
    lg64 = logits.astype(np.float64)
    m = lg64.max(axis=1, keepdims=True)
    lse = (m[:, 0] + np.log(np.exp(lg64 - m).sum(axis=1)))
    nll = lse - lg64[np.arange(_B), labels.astype(np.int64)]
    loss = np.float32(nll.mean())
    return loss, logits
